# revision 1
# baseline (speedup 1.0000x reference)
"""HFreqC layer kernel for 8 Trainium2 NeuronCores.

The reference op (FFT -> zero centered low-freq band -> IFFT -> real -> relu)
is, up to the relu, a fixed real linear operator along the channel axis:
    y = relu(x @ W),  W = Re(ifft(mask * fft(I)))^T   (728x728, symmetric)

Strategy: pure data parallel over rows (32*38*38 = 46208 rows, 5776/core,
padded to 6144 = 12 groups of 512 rows). The host shards rows across the 8
cores and lays each shard out channel-major (transposed) while padding, so
the device reads are all contiguous. Per core:
  - W (row-padded to 768) lives in SBUF as 6 k-tiles [128, 728].
  - Each 512-row group is one contiguous [128, 3072] DMA holding X^T tiles
    [128ch x 128row] for (g in 4 row-tiles) x (u in 6 k-tiles).
  - fp32 matmuls in float32r mode (1 cycle/row at N>=256), accumulating
    over 6 k-tiles into PSUM, j in two 364-wide chunks.
  - ScalarE applies relu on the PSUM->SBUF copy; contiguous DMA out.
"""

import numpy as np

C = 728            # channels
KT = 6             # k tiles of 128 (channel pad to 768)
CP = KT * 128      # 768 padded channels
G = 4              # row-tiles (128 rows) per group
GROUP_ROWS = 128 * G
N_CORES = 8
ROWS_TOTAL = 32 * 38 * 38          # 46208
ROWS_PER_CORE = ROWS_TOTAL // N_CORES  # 5776
N_GROUPS = 12
ROWS_PAD = N_GROUPS * GROUP_ROWS   # 6144
JC = 364           # j-chunk width (2 chunks of 364; both >=256 for f32r rate)

_CACHE = {}


def _build_w(scale: int) -> np.ndarray:
    """[CP, C] f32: W padded with zero rows; y_row = x_row @ W."""
    m_sh = np.ones(C)
    m_sh[C // 2 - C // scale: C // 2 + C // scale] = 0
    m = np.fft.ifftshift(m_sh)
    A = np.fft.ifft(m[:, None] * np.fft.fft(np.eye(C), axis=0), axis=0)
    W = np.real(A).T.astype(np.float32)
    Wp = np.zeros((CP, C), dtype=np.float32)
    Wp[:C] = W
    return Wp


def _shard_xt(xf: np.ndarray, core: int) -> np.ndarray:
    """[N_GROUPS, 128, G*CP]: [grp][p][g*CP + u*128 + m] = x[512grp+128g+m, 128u+p]."""
    xp = np.zeros((ROWS_PAD, CP), dtype=np.float32)
    xp[:ROWS_PER_CORE, :C] = xf[core * ROWS_PER_CORE:(core + 1) * ROWS_PER_CORE]
    v = xp.reshape(N_GROUPS, G, 128, KT, 128)          # grp g m u p
    v = v.transpose(0, 4, 1, 3, 2)                     # grp p g u m
    return np.ascontiguousarray(v).reshape(N_GROUPS, 128, G * CP)


def _build_nc(repeat: int = 1):
    import concourse.mybir as mybir
    import concourse.tile as tile
    from concourse import bacc

    fp32 = mybir.dt.float32
    fp32r = mybir.dt.float32r

    nc = bacc.Bacc("TRN2", target_bir_lowering=False)
    x_d = nc.dram_tensor("x", [N_GROUPS, 128, G * CP], fp32r, kind="ExternalInput").ap()
    w_d = nc.dram_tensor("w", [CP, C], fp32r, kind="ExternalInput").ap()
    y_d = nc.dram_tensor("y", [ROWS_PAD, C], fp32, kind="ExternalOutput").ap()

    y_v = y_d.rearrange("(grp g p) j -> grp p g j", g=G, p=128)
    w_v = w_d.rearrange("(u p) j -> p u j", u=KT, p=128)

    with tile.TileContext(nc) as tc:
        with (
            tc.tile_pool(name="wpool", bufs=1) as wpool,
            tc.tile_pool(name="io", bufs=4) as io,
            tc.tile_pool(name="psp", bufs=8, space="PSUM") as psp,
        ):
            w_sb = wpool.tile([128, KT * C], fp32r)
            nc.sync.dma_start(out=w_sb.rearrange("p (u j) -> p u j", u=KT, j=C), in_=w_v)
            for _it in range(N_GROUPS * repeat):
                grp = _it % N_GROUPS
                xt = io.tile([128, G * CP], fp32r, tag="xt")
                half = G * CP // 2
                nc.sync.dma_start(out=xt[:, :half], in_=x_d[grp, :, :half])
                nc.sync.dma_start(out=xt[:, half:], in_=x_d[grp, :, half:])
                ysb = io.tile([128, G * C], fp32, tag="y")
                for g in range(G):
                    for jc in range(2):
                        j0 = jc * JC
                        ps = psp.tile([128, JC], fp32, tag="ps")
                        for u in range(KT):
                            nc.tensor.matmul(
                                ps,
                                lhsT=xt[:, g * CP + u * 128: g * CP + (u + 1) * 128],
                                rhs=w_sb[:, u * C + j0: u * C + j0 + JC],
                                start=(u == 0),
                                stop=(u == KT - 1),
                            )
                        nc.scalar.activation(
                            ysb[:, g * C + j0: g * C + j0 + JC],
                            ps,
                            mybir.ActivationFunctionType.Relu,
                        )
                ysb_v = ysb.rearrange("p (g j) -> p g j", g=G, j=C)
                nc.scalar.dma_start(out=y_v[grp][:, 0:2], in_=ysb_v[:, 0:2])
                nc.scalar.dma_start(out=y_v[grp][:, 2:4], in_=ysb_v[:, 2:4])
    nc.compile()
    return nc


def _make_in_maps(x: np.ndarray, scale: int):
    xf = np.asarray(x, dtype=np.float32).reshape(-1, C)
    W = _build_w(scale)
    return [{"x": _shard_xt(xf, i), "w": W} for i in range(N_CORES)]


def kernel(x: np.ndarray, scale) -> np.ndarray:
    import sys
    if "/opt/trn_rl_repo" not in sys.path:
        sys.path.insert(0, "/opt/trn_rl_repo")
    from concourse.bass_utils import run_bass_kernel_spmd

    scale = int(np.asarray(scale))
    x = np.asarray(x, dtype=np.float32)
    orig_shape = x.shape

    if "nc" not in _CACHE:
        _CACHE["nc"] = _build_nc()
    nc = _CACHE["nc"]

    in_maps = _make_in_maps(x, scale)
    res = run_bass_kernel_spmd(nc, in_maps, list(range(N_CORES)))
    outs = [r["y"][:ROWS_PER_CORE] for r in res.results]
    y = np.concatenate(outs, axis=0).reshape(orig_shape)
    return y.astype(np.float32)



# revision 11
# speedup vs baseline: 39.4480x; 39.4480x over previous
"""HFreqC layer kernel for 8 Trainium2 NeuronCores.

The reference op (FFT -> zero centered low-freq band -> IFFT -> real -> relu)
is, up to the relu, a fixed real linear operator along the channel axis:
    y = relu(x @ W),  W = Re(ifft(mask * fft(I)))^T   (728x728, symmetric)

Strategy: pure data parallel over rows (32*38*38 = 46208 rows). Each core
processes 46 row-tiles of 128 rows (5888 rows; last core is zero-padded).
All device I/O is bf16 (quantization adds ~0.2% rel err vs the 2e-2 gate),
which halves HBM traffic and makes the kernel TensorE-bound:
  - W (row-padded to 768) lives in SBUF as bf16 [128, 6*728].
  - Per row-tile one contiguous [128, 768] bf16 DMA holds the 6 k-tiles of
    X^T (channel-major: [t][p][u*128+m] = x[t*128+m, u*128+p]).
  - bf16 matmuls accumulate over 6 k-tiles into PSUM, j in two 364 chunks.
  - ScalarE applies relu on the PSUM->SBUF copy, casting to bf16.
  - One contiguous [128, 728] bf16 DMA out per row-tile.
Engine budget per row-tile: PE 12x364cyc ~ 1820ns (bound), SP in+out DMA
~1153ns, ACT 2 activations ~980ns.
"""

import numpy as np

C = 728            # channels
KT = 6             # k tiles of 128 (channel pad to 768)
CP = KT * 128      # 768 padded channels
N_CORES = 8
ROWS_TOTAL = 32 * 38 * 38          # 46208
N_TILES = 46                       # 128-row tiles per core
ROWS_PER_CORE = N_TILES * 128      # 5888 (padded; 8*5888 = 47104 >= 46208)
JC = 364           # j-chunk width (2 chunks of 364; psum bank holds 512 f32)

_CACHE = {}


def _f32_to_bf16_u16(a: np.ndarray) -> np.ndarray:
    """Round-to-nearest-even f32 -> bf16, as uint16 payload (fast, vectorized)."""
    u = a.view(np.uint32)
    rounded = u + np.uint32(0x7FFF) + ((u >> np.uint32(16)) & np.uint32(1))
    return (rounded >> np.uint32(16)).astype(np.uint16)


def _bf16_u16_to_f32(u: np.ndarray) -> np.ndarray:
    return (u.astype(np.uint32) << np.uint32(16)).view(np.float32)


def _bf16(a: np.ndarray):
    import ml_dtypes
    return _f32_to_bf16_u16(np.ascontiguousarray(a)).view(ml_dtypes.bfloat16)


def _build_w(scale: int) -> np.ndarray:
    """[CP, C] f32: W padded with zero rows; y_row = x_row @ W."""
    m_sh = np.ones(C)
    m_sh[C // 2 - C // scale: C // 2 + C // scale] = 0
    m = np.fft.ifftshift(m_sh)
    A = np.fft.ifft(m[:, None] * np.fft.fft(np.eye(C), axis=0), axis=0)
    W = np.real(A).T.astype(np.float32)
    Wp = np.zeros((CP, C), dtype=np.float32)
    Wp[:C] = W
    return Wp


def _shard_xt(xf: np.ndarray, core: int) -> np.ndarray:
    """[N_TILES, 128, CP] bf16: [t][p][u*128+m] = x[t*128+m, 128u+p]."""
    lo = core * ROWS_PER_CORE
    hi = min(lo + ROWS_PER_CORE, ROWS_TOTAL)
    xp = np.zeros((ROWS_PER_CORE, CP), dtype=np.uint16)
    xp[:hi - lo, :C] = _f32_to_bf16_u16(xf[lo:hi])
    v = xp.reshape(N_TILES, 128, KT, 128)              # t m u p
    v = v.transpose(0, 3, 2, 1)                        # t p u m
    import ml_dtypes
    return np.ascontiguousarray(v).reshape(N_TILES, 128, CP).view(ml_dtypes.bfloat16)


def _build_nc(repeat: int = 1):
    """One full pass over the core's shard; repeat>1 wraps it in a HW loop
    (used only for steady-state timing -- same data is reprocessed)."""
    import concourse.mybir as mybir
    import concourse.tile as tile
    from concourse import bacc
    from contextlib import nullcontext

    fp32 = mybir.dt.float32
    bf16 = mybir.dt.bfloat16

    nc = bacc.Bacc("TRN2", target_bir_lowering=False)
    x_d = nc.dram_tensor("x", [N_TILES, 128, CP], bf16, kind="ExternalInput").ap()
    w_d = nc.dram_tensor("w", [CP, C], bf16, kind="ExternalInput").ap()
    y_d = nc.dram_tensor("y", [N_TILES, 128, C], bf16, kind="ExternalOutput").ap()

    w_v = w_d.rearrange("(u p) j -> p u j", u=KT, p=128)

    with tile.TileContext(nc) as tc:
        with (
            tc.tile_pool(name="wpool", bufs=1) as wpool,
            tc.tile_pool(name="io", bufs=4) as io,
            tc.tile_pool(name="psp", bufs=6, space="PSUM") as psp,
        ):
            # Split the W load per k-tile so the first matmul only waits for
            # chunk u=0 (~2.3us) instead of the full 3.4us transfer.
            w_tiles = [wpool.tile([128, C], bf16, name=f"w{u}") for u in range(KT)]
            for u in range(KT):
                nc.scalar.dma_start(out=w_tiles[u], in_=w_v[:, u])


            def one_pass():
                for t in range(N_TILES):
                    xt = io.tile([128, CP], bf16, tag="xt")
                    nc.sync.dma_start(out=xt, in_=x_d[t])
                    ysb = io.tile([128, C], bf16, tag="y")
                    for jc in range(2):
                        j0 = jc * JC
                        ps = psp.tile([128, JC], fp32, tag="ps")
                        for u in range(KT):
                            nc.tensor.matmul(
                                ps,
                                lhsT=xt[:, u * 128:(u + 1) * 128],
                                rhs=w_tiles[u][:, j0:j0 + JC],
                                start=(u == 0),
                                stop=(u == KT - 1),
                            )
                        nc.scalar.activation(
                            ysb[:, j0:j0 + JC],
                            ps,
                            mybir.ActivationFunctionType.Relu,
                        )
                        nc.sync.dma_start(out=y_d[t][:, j0:j0 + JC],
                                          in_=ysb[:, j0:j0 + JC])

            if repeat == 1:
                one_pass()
            else:
                import concourse.mybir as _mb
                with tc.For_i(0, repeat, 1,
                              hint_engines=(_mb.EngineType.PE,)):
                    one_pass()
    nc.compile()
    return nc


def _make_in_maps(x: np.ndarray, scale: int):
    xf = np.asarray(x, dtype=np.float32).reshape(-1, C)
    W = _bf16(_build_w(scale))
    return [{"x": _shard_xt(xf, i), "w": W} for i in range(N_CORES)]


def kernel(x: np.ndarray, scale) -> np.ndarray:
    import sys
    if "/opt/trn_rl_repo" not in sys.path:
        sys.path.insert(0, "/opt/trn_rl_repo")
    from concourse.bass_utils import run_bass_kernel_spmd

    scale = int(np.asarray(scale))
    x = np.asarray(x, dtype=np.float32)
    orig_shape = x.shape

    if "nc" not in _CACHE:
        _CACHE["nc"] = _build_nc()
    nc = _CACHE["nc"]

    in_maps = _make_in_maps(x, scale)
    res = run_bass_kernel_spmd(nc, in_maps, list(range(N_CORES)))
    outs = []
    for i, r in enumerate(res.results):
        lo = i * ROWS_PER_CORE
        hi = min(lo + ROWS_PER_CORE, ROWS_TOTAL)
        yb = np.asarray(r["y"]).reshape(ROWS_PER_CORE, C)[:hi - lo]
        outs.append(_bf16_u16_to_f32(yb.view(np.uint16)))
    y = np.concatenate(outs, axis=0).reshape(orig_shape)
    return y.astype(np.float32)


# revision 13
# speedup vs baseline: 41.9350x; 1.0630x over previous
"""HFreqC layer kernel for 8 Trainium2 NeuronCores.

The reference op (FFT -> zero centered low-freq band -> IFFT -> real -> relu)
is, up to the relu, a fixed real linear operator along the channel axis:
    y = relu(x @ W),  W = Re(ifft(mask * fft(I)))^T   (728x728, symmetric)

Strategy: pure data parallel over rows (32*38*38 = 46208 rows). Each core
processes 46 row-tiles of 128 rows (5888 rows; last core is zero-padded).
All device I/O is bf16 (quantization adds ~0.2% rel err vs the 2e-2 gate),
which halves HBM traffic and makes the kernel TensorE-bound:
  - W (row-padded to 768) lives in SBUF as bf16 [128, 6*728].
  - Per row-tile one contiguous [128, 768] bf16 DMA holds the 6 k-tiles of
    X^T (channel-major: [t][p][u*128+m] = x[t*128+m, u*128+p]).
  - bf16 matmuls accumulate over 6 k-tiles into PSUM, j in two 364 chunks.
  - ScalarE applies relu on the PSUM->SBUF copy, casting to bf16.
  - One contiguous [128, 728] bf16 DMA out per row-tile.
Engine budget per row-tile: PE 12x364cyc ~ 1820ns (bound), SP in+out DMA
~1153ns, ACT 2 activations ~980ns.
"""

import numpy as np

C = 728            # channels
KT = 6             # k tiles of 128 (channel pad to 768)
CP = KT * 128      # 768 padded channels
N_CORES = 8
ROWS_TOTAL = 32 * 38 * 38          # 46208
N_TILES = 46                       # 128-row tiles per core
ROWS_PER_CORE = N_TILES * 128      # 5888 (padded; 8*5888 = 47104 >= 46208)
JC = 364           # j-chunk width (2 chunks of 364; psum bank holds 512 f32)

_CACHE = {}


def _f32_to_bf16_u16(a: np.ndarray) -> np.ndarray:
    """Round-to-nearest-even f32 -> bf16, as uint16 payload (fast, vectorized)."""
    u = a.view(np.uint32)
    rounded = u + np.uint32(0x7FFF) + ((u >> np.uint32(16)) & np.uint32(1))
    return (rounded >> np.uint32(16)).astype(np.uint16)


def _bf16_u16_to_f32(u: np.ndarray) -> np.ndarray:
    return (u.astype(np.uint32) << np.uint32(16)).view(np.float32)


def _bf16(a: np.ndarray):
    import ml_dtypes
    return _f32_to_bf16_u16(np.ascontiguousarray(a)).view(ml_dtypes.bfloat16)


def _build_w(scale: int) -> np.ndarray:
    """[CP, C] f32: W padded with zero rows; y_row = x_row @ W."""
    m_sh = np.ones(C)
    m_sh[C // 2 - C // scale: C // 2 + C // scale] = 0
    m = np.fft.ifftshift(m_sh)
    A = np.fft.ifft(m[:, None] * np.fft.fft(np.eye(C), axis=0), axis=0)
    W = np.real(A).T.astype(np.float32)
    Wp = np.zeros((CP, C), dtype=np.float32)
    Wp[:C] = W
    return Wp


def _shard_xt(xf: np.ndarray, core: int) -> np.ndarray:
    """[N_TILES, 128, CP] bf16: [t][p][u*128+m] = x[t*128+m, 128u+p]."""
    lo = core * ROWS_PER_CORE
    hi = min(lo + ROWS_PER_CORE, ROWS_TOTAL)
    xp = np.zeros((ROWS_PER_CORE, CP), dtype=np.uint16)
    xp[:hi - lo, :C] = _f32_to_bf16_u16(xf[lo:hi])
    v = xp.reshape(N_TILES, 128, KT, 128)              # t m u p
    v = v.transpose(0, 3, 2, 1)                        # t p u m
    import ml_dtypes
    return np.ascontiguousarray(v).reshape(N_TILES, 128, CP).view(ml_dtypes.bfloat16)


def _build_nc(repeat: int = 1):
    """One full pass over the core's shard; repeat>1 wraps it in a HW loop
    (used only for steady-state timing -- same data is reprocessed)."""
    import concourse.mybir as mybir
    import concourse.tile as tile
    from concourse import bacc
    from contextlib import nullcontext

    fp32 = mybir.dt.float32
    bf16 = mybir.dt.bfloat16

    nc = bacc.Bacc("TRN2", target_bir_lowering=False)
    x_d = nc.dram_tensor("x", [N_TILES, 128, CP], bf16, kind="ExternalInput").ap()
    w_d = nc.dram_tensor("w", [CP, C], bf16, kind="ExternalInput").ap()
    y_d = nc.dram_tensor("y", [N_TILES, 128, C], bf16, kind="ExternalOutput").ap()

    w_v = w_d.rearrange("(u p) j -> p u j", u=KT, p=128)

    with tile.TileContext(nc) as tc:
        with (
            tc.tile_pool(name="wpool", bufs=1) as wpool,
            tc.tile_pool(name="io", bufs=4) as io,
            tc.tile_pool(name="psp", bufs=6, space="PSUM") as psp,
        ):
            # Split the W load per k-tile so the first matmul only waits for
            # chunk u=0 (~2.3us) instead of the full 3.4us transfer.
            w_tiles = [wpool.tile([128, C], bf16, name=f"w{u}") for u in range(KT)]
            for u in range(KT):
                nc.scalar.dma_start(out=w_tiles[u], in_=w_v[:, u])


            def one_pass():
                for t in range(N_TILES):
                    xt = io.tile([128, CP], bf16, tag="xt")
                    nc.sync.dma_start(out=xt, in_=x_d[t])
                    ysb = io.tile([128, C], bf16, tag="y")
                    for jc in range(2):
                        j0 = jc * JC
                        ps = psp.tile([128, JC], fp32, tag="ps")
                        for u in range(KT):
                            nc.tensor.matmul(
                                ps,
                                lhsT=xt[:, u * 128:(u + 1) * 128],
                                rhs=w_tiles[u][:, j0:j0 + JC],
                                start=(u == 0),
                                stop=(u == KT - 1),
                            )
                        nc.scalar.activation(
                            ysb[:, j0:j0 + JC],
                            ps,
                            mybir.ActivationFunctionType.Relu,
                        )
                    nc.scalar.dma_start(out=y_d[t], in_=ysb)

            if repeat == 1:
                one_pass()
            else:
                import concourse.mybir as _mb
                with tc.For_i(0, repeat, 1,
                              hint_engines=(_mb.EngineType.PE,)):
                    one_pass()
    nc.compile()
    return nc


def _make_in_maps(x: np.ndarray, scale: int):
    xf = np.asarray(x, dtype=np.float32).reshape(-1, C)
    W = _bf16(_build_w(scale))
    return [{"x": _shard_xt(xf, i), "w": W} for i in range(N_CORES)]


def kernel(x: np.ndarray, scale) -> np.ndarray:
    import sys
    if "/opt/trn_rl_repo" not in sys.path:
        sys.path.insert(0, "/opt/trn_rl_repo")
    from concourse.bass_utils import run_bass_kernel_spmd

    scale = int(np.asarray(scale))
    x = np.asarray(x, dtype=np.float32)
    orig_shape = x.shape

    if "nc" not in _CACHE:
        _CACHE["nc"] = _build_nc()
    nc = _CACHE["nc"]

    in_maps = _make_in_maps(x, scale)
    res = run_bass_kernel_spmd(nc, in_maps, list(range(N_CORES)))
    outs = []
    for i, r in enumerate(res.results):
        lo = i * ROWS_PER_CORE
        hi = min(lo + ROWS_PER_CORE, ROWS_TOTAL)
        yb = np.asarray(r["y"]).reshape(ROWS_PER_CORE, C)[:hi - lo]
        outs.append(_bf16_u16_to_f32(yb.view(np.uint16)))
    y = np.concatenate(outs, axis=0).reshape(orig_shape)
    return y.astype(np.float32)


# revision 15
# speedup vs baseline: 42.1420x; 1.0049x over previous
"""HFreqC layer kernel for 8 Trainium2 NeuronCores.

The reference op (FFT -> zero centered low-freq band -> IFFT -> real -> relu)
is, up to the relu, a fixed real linear operator along the channel axis:
    y = relu(x @ W),  W = Re(ifft(mask * fft(I)))^T   (728x728, symmetric)

Strategy: pure data parallel over rows (32*38*38 = 46208 rows). Each core
processes 46 row-tiles of 128 rows (5888 rows; last core is zero-padded).
All device I/O is bf16 (quantization adds ~0.2% rel err vs the 2e-2 gate),
which halves HBM traffic and makes the kernel TensorE-bound:
  - W (row-padded to 768) lives in SBUF as bf16 [128, 6*728].
  - Per row-tile one contiguous [128, 768] bf16 DMA holds the 6 k-tiles of
    X^T (channel-major: [t][p][u*128+m] = x[t*128+m, u*128+p]).
  - bf16 matmuls accumulate over 6 k-tiles into PSUM, j in two 364 chunks.
  - ScalarE applies relu on the PSUM->SBUF copy, casting to bf16.
  - One contiguous [128, 728] bf16 DMA out per row-tile.
Engine budget per row-tile: PE 12x364cyc ~ 1820ns (bound), SP in+out DMA
~1153ns, ACT 2 activations ~980ns.
"""

import numpy as np

C = 728            # channels
KT = 6             # k tiles of 128 (channel pad to 768)
CP = KT * 128      # 768 padded channels
N_CORES = 8
ROWS_TOTAL = 32 * 38 * 38          # 46208
N_TILES = 46                       # 128-row tiles per core
ROWS_PER_CORE = N_TILES * 128      # 5888 (padded; 8*5888 = 47104 >= 46208)
JC = 364           # j-chunk width (2 chunks of 364; psum bank holds 512 f32)

_CACHE = {}


def _f32_to_bf16_u16(a: np.ndarray) -> np.ndarray:
    """Round-to-nearest-even f32 -> bf16, as uint16 payload (fast, vectorized)."""
    u = a.view(np.uint32)
    rounded = u + np.uint32(0x7FFF) + ((u >> np.uint32(16)) & np.uint32(1))
    return (rounded >> np.uint32(16)).astype(np.uint16)


def _bf16_u16_to_f32(u: np.ndarray) -> np.ndarray:
    return (u.astype(np.uint32) << np.uint32(16)).view(np.float32)


def _bf16(a: np.ndarray):
    import ml_dtypes
    return _f32_to_bf16_u16(np.ascontiguousarray(a)).view(ml_dtypes.bfloat16)


def _build_w(scale: int) -> np.ndarray:
    """[CP, C] f32: W padded with zero rows; y_row = x_row @ W."""
    m_sh = np.ones(C)
    m_sh[C // 2 - C // scale: C // 2 + C // scale] = 0
    m = np.fft.ifftshift(m_sh)
    A = np.fft.ifft(m[:, None] * np.fft.fft(np.eye(C), axis=0), axis=0)
    W = np.real(A).T.astype(np.float32)
    Wp = np.zeros((CP, C), dtype=np.float32)
    Wp[:C] = W
    return Wp


def _shard_xt(xf: np.ndarray, core: int) -> np.ndarray:
    """[N_TILES, 128, CP] bf16: [t][p][u*128+m] = x[t*128+m, 128u+p]."""
    lo = core * ROWS_PER_CORE
    hi = min(lo + ROWS_PER_CORE, ROWS_TOTAL)
    xp = np.zeros((ROWS_PER_CORE, CP), dtype=np.uint16)
    xp[:hi - lo, :C] = _f32_to_bf16_u16(xf[lo:hi])
    v = xp.reshape(N_TILES, 128, KT, 128)              # t m u p
    v = v.transpose(0, 3, 2, 1)                        # t p u m
    import ml_dtypes
    return np.ascontiguousarray(v).reshape(N_TILES, 128, CP).view(ml_dtypes.bfloat16)


def _build_nc(repeat: int = 1):
    """One full pass over the core's shard; repeat>1 wraps it in a HW loop
    (used only for steady-state timing -- same data is reprocessed)."""
    import concourse.mybir as mybir
    import concourse.tile as tile
    from concourse import bacc
    from contextlib import nullcontext

    fp32 = mybir.dt.float32
    bf16 = mybir.dt.bfloat16

    nc = bacc.Bacc("TRN2", target_bir_lowering=False)
    x_d = nc.dram_tensor("x", [N_TILES, 128, CP], bf16, kind="ExternalInput").ap()
    w_d = nc.dram_tensor("w", [CP, C], bf16, kind="ExternalInput").ap()
    y_d = nc.dram_tensor("y", [N_TILES, 128, C], bf16, kind="ExternalOutput").ap()

    w_v = w_d.rearrange("(u p) j -> p u j", u=KT, p=128)

    with tile.TileContext(nc) as tc:
        with (
            tc.tile_pool(name="wpool", bufs=1) as wpool,
            tc.tile_pool(name="io", bufs=8) as io,
            tc.tile_pool(name="psp", bufs=6, space="PSUM") as psp,
        ):
            # Split the W load per k-tile so the first matmul only waits for
            # chunk u=0 (~2.3us) instead of the full 3.4us transfer.
            w_tiles = [wpool.tile([128, C], bf16, name=f"w{u}") for u in range(KT)]
            for u in range(KT):
                nc.scalar.dma_start(out=w_tiles[u], in_=w_v[:, u])


            def one_pass():
                for t in range(N_TILES):
                    xt = io.tile([128, CP], bf16, tag="xt")
                    nc.sync.dma_start(out=xt, in_=x_d[t])
                    ysb = io.tile([128, C], bf16, tag="y")
                    for jc in range(2):
                        j0 = jc * JC
                        ps = psp.tile([128, JC], fp32, tag="ps")
                        for u in range(KT):
                            nc.tensor.matmul(
                                ps,
                                lhsT=xt[:, u * 128:(u + 1) * 128],
                                rhs=w_tiles[u][:, j0:j0 + JC],
                                start=(u == 0),
                                stop=(u == KT - 1),
                            )
                        nc.scalar.activation(
                            ysb[:, j0:j0 + JC],
                            ps,
                            mybir.ActivationFunctionType.Relu,
                        )
                    nc.scalar.dma_start(out=y_d[t], in_=ysb)

            if repeat == 1:
                one_pass()
            else:
                import concourse.mybir as _mb
                with tc.For_i(0, repeat, 1,
                              hint_engines=(_mb.EngineType.PE,),
                              staggered_reset=True):
                    one_pass()
    nc.compile()
    return nc


def _make_in_maps(x: np.ndarray, scale: int):
    xf = np.asarray(x, dtype=np.float32).reshape(-1, C)
    W = _bf16(_build_w(scale))
    return [{"x": _shard_xt(xf, i), "w": W} for i in range(N_CORES)]


def kernel(x: np.ndarray, scale) -> np.ndarray:
    import sys
    if "/opt/trn_rl_repo" not in sys.path:
        sys.path.insert(0, "/opt/trn_rl_repo")
    from concourse.bass_utils import run_bass_kernel_spmd

    scale = int(np.asarray(scale))
    x = np.asarray(x, dtype=np.float32)
    orig_shape = x.shape

    if "nc" not in _CACHE:
        _CACHE["nc"] = _build_nc()
    nc = _CACHE["nc"]

    in_maps = _make_in_maps(x, scale)
    res = run_bass_kernel_spmd(nc, in_maps, list(range(N_CORES)))
    outs = []
    for i, r in enumerate(res.results):
        lo = i * ROWS_PER_CORE
        hi = min(lo + ROWS_PER_CORE, ROWS_TOTAL)
        yb = np.asarray(r["y"]).reshape(ROWS_PER_CORE, C)[:hi - lo]
        outs.append(_bf16_u16_to_f32(yb.view(np.uint16)))
    y = np.concatenate(outs, axis=0).reshape(orig_shape)
    return y.astype(np.float32)
